# revision 37
# baseline (speedup 1.0000x reference)
"""AttentionBlock (GroupNorm -> QKV -> 8-head attention -> proj -> residual)
as a Bass/Tile kernel for Trainium2, data-parallel over batch on 8 cores.

Self-contained: hardcodes shapes B=8, C=512, H=W=32 (N=1024), heads=8, d=64,
groups=32.  Each core processes one batch element; all params replicated.

Key structure (v2 — globally software-pipelined):
  x [C, N] channel-major -> 4 SBUF tiles [128, 1024].
  GroupNorm: per-channel mean/var via bn_stats/bn_aggr, cross-partition group
  aggregation + broadcast via two tiny mask matmuls on the PE.
  QKV: only Q (m-tiles 0-3) and K-padded (m-tiles 4-11; head h occupies
  rows (h%2)*64 of tile 4+h, other rows zero so the K=128 contraction is
  head-exclusive).  V never materializes channel-major: vT[keys, 8*64] is
  computed directly as xn^T @ wvT (4-step chains per key tile), drained into
  bf16 vt tiles [128, 8, 128] whose odd 64-col halves are pre-memset to 1.0
  (gpsimd) — the ones block makes the context matmul broadcast the softmax
  denominator into output rows 64-127 for free.
  Biases: Q bias kept; K bias dropped (exactly softmax-invariant: it only
  adds per-query constants to scores); V bias folded into the proj bias on
  the host (proj_b + proj_w @ b_v, valid because softmax rows sum to 1).
  Attention per head pair (transposed orientation, no max-subtraction):
  scoresT = K^T Q on the PE, exp on ACT (scale=1/8) -> bf16 probs,
  contextT accumulated as vt^T @ probsT.  1/denominator via DVE reciprocal
  (NOT Ln/Exp on ACT — saves ~27us of ACT incl. table switches), multiply on
  DVE -> h_attT tiles.  proj: wprojT.T @ h_attT + bias' + x -> out.

Scheduling: the exp stream on ACT (64 x [128,1024] tiles ~ 67us) is the
critical resource; it is started as early as possible and kept fed.  PE
program order: GN mms -> QKV m-tiles for pairs 0-1 -> vT(kt=0,1) -> pair 0
(injecting vT(2..7) chains into its stream) -> pair 1 (injecting pair 2's
QKV chains) -> pair 2 (injecting pair 3's) -> pair 3 -> proj.  Injected
chains reuse the score-slot PSUM banks (tag sharing) at pair head/tail
where the exp pipeline covers them.  The Exp LUT is preloaded by a dummy
activation during the QKV phase so the first real exp pays no table load.
PSUM: scores 2 slots x 2 banks, context 2 slots x 2 banks = 8 banks.

Matmul inputs are float32r (1 cycle/row for moving free >= 256; fp32 would
be 4) except probs/vt which are bf16.  f32r operands must be *produced* as
f32r, so every tile feeding a matmul is allocated f32r.
"""

import sys

sys.path.insert(0, "/opt/trn_rl_repo")

import numpy as np

B, C, HH, WW = 8, 512, 32, 32
N = HH * WW          # 1024
NH, HD = 8, 64       # heads, head dim
NG = 32              # groupnorm groups
EPS = 1e-5
NT = C // 128        # 4 channel tiles
MT = 12              # qkv m-tiles: Q 0-3 | K-padded 4-11
KT = N // 128        # 8 key tiles
NCORES = 8
LAG = 2              # context matmuls run LAG k-tiles behind scores/exp

_CACHE: dict = {}


def _build_program():
    import concourse.bacc as bacc
    import concourse.tile as tile
    from concourse import mybir

    f32 = mybir.dt.float32
    f32r = mybir.dt.float32r
    bf16 = mybir.dt.bfloat16
    AF = mybir.ActivationFunctionType
    OP = mybir.AluOpType

    nc = bacc.Bacc("TRN2", target_bir_lowering=False, debug=False)
    # We place activation-table loads by hand (sqrt set before the GN sqrt,
    # then the combined exp+ln set once for the whole attention phase).  The
    # automatic pass does not track hand-placed loads and would re-insert a
    # single-function set load at every exp<->ln transition (8 switches,
    # ~1.3us each), so disable it for this program.
    nc.insert_act_table_loads = lambda: None

    x_d = nc.dram_tensor("x", [C, N], f32, kind="ExternalInput").ap()
    # weights in bf16: halves the weight DMA traffic and doubles the PE
    # ldweights rate (bf16 loads 1 row/cycle vs ~2.5 for f32r); the moving
    # operands stay f32r so matmuls still stream at 1 cycle/row.
    wqkv_d = nc.dram_tensor("wqkvT", [C, MT * 128], bf16, kind="ExternalInput").ap()
    wv_d = nc.dram_tensor("wvT", [C, C], bf16, kind="ExternalInput").ap()
    wproj_d = nc.dram_tensor("wprojT", [C, C], bf16, kind="ExternalInput").ap()
    bqkv_d = nc.dram_tensor("bqkv", [128, MT], f32, kind="ExternalInput").ap()
    bproj_d = nc.dram_tensor("bproj", [128, NT], f32, kind="ExternalInput").ap()
    gnw_d = nc.dram_tensor("gnw", [128, NT], f32, kind="ExternalInput").ap()
    gnb_d = nc.dram_tensor("gnb", [128, NT], f32, kind="ExternalInput").ap()
    gmask_d = nc.dram_tensor("gmask", [128, 8], f32, kind="ExternalInput").ap()
    gmaskT_d = nc.dram_tensor("gmaskT", [8, 128], f32, kind="ExternalInput").ap()
    out_d = nc.dram_tensor("out", [C, N], f32, kind="ExternalOutput").ap()

    x_dt = x_d.rearrange("(t p) n -> t p n", p=128)
    out_dt = out_d.rearrange("(t p) n -> t p n", p=128)
    wq_dt = wqkv_d.rearrange("(t p) m -> t p m", p=128)
    wv_dt = wv_d.rearrange("(t p) m -> t p m", p=128)
    wp_dt = wproj_d.rearrange("(t p) m -> t p m", p=128)

    from contextlib import ExitStack

    with tile.TileContext(nc) as tc, ExitStack() as ctx:
        sg = ctx.enter_context(tc.tile_pool(name="sg", bufs=1))
        work = ctx.enter_context(tc.tile_pool(name="work", bufs=1))
        pb_pool = ctx.enter_context(tc.tile_pool(name="pbp", bufs=8))
        small = ctx.enter_context(tc.tile_pool(name="small", bufs=4))
        outp = ctx.enter_context(tc.tile_pool(name="outp", bufs=2))
        # PSUM budget (8 banks): "sc" slots 2x2 banks (score tiles; shared by
        # the QKV/vT/proj half-accumulator chains and the GN matmuls via tag
        # reuse), "cx" slots 2x2 banks (context accumulators of a pair; also
        # chain slots outside attention).
        psc = ctx.enter_context(tc.tile_pool(name="psc", bufs=2, space="PSUM"))
        pcx = ctx.enter_context(tc.tile_pool(name="pcx", bufs=2, space="PSUM"))

        # ---- x first (critical path for GN): quarter-granular DMAs so
        # bn_stats can start on each quarter as it lands ----
        x_sb = []
        for t in range(NT):
            xt = work.tile([128, N], f32, name=f"x{t}", tag=f"x{t}")
            for q in range(4):
                nc.sync.dma_start(
                    out=xt[:, q * 256 : (q + 1) * 256],
                    in_=x_dt[t][:, q * 256 : (q + 1) * 256],
                )
            x_sb.append(xt)

        # ---- small constants (biases needed at first drains) ----
        bqkv_sb = sg.tile([128, MT], f32, name="bqkv_sb")
        nc.sync.dma_start(out=bqkv_sb, in_=bqkv_d)
        bproj_sb = sg.tile([128, NT], f32, name="bproj_sb")
        nc.sync.dma_start(out=bproj_sb, in_=bproj_d)
        gnw_sb = sg.tile([128, NT], f32, name="gnw_sb")
        nc.sync.dma_start(out=gnw_sb, in_=gnw_d)
        gnb_sb = sg.tile([128, NT], f32, name="gnb_sb")
        nc.sync.dma_start(out=gnb_sb, in_=gnb_d)
        gmask_sb = sg.tile([128, 8], f32, name="gmask_sb")
        nc.sync.dma_start(out=gmask_sb, in_=gmask_d)
        gmaskT_sb = sg.tile([8, 128], f32, name="gmaskT_sb")
        nc.sync.dma_start(out=gmaskT_sb, in_=gmaskT_d)

        # ---- weights, ordered by first use ----
        # pre-phase m-tiles {0,4,5,1,6,7}, then wvT, then the rest, wproj last
        wq_sb = []
        for t in range(NT):
            wt = sg.tile([128, MT * 128], bf16, name=f"wq{t}", tag=f"wq{t}")
            wq_sb.append(wt)
        for m in (0, 4, 5, 1, 6, 7):
            for t in range(NT):
                nc.sync.dma_start(
                    out=wq_sb[t][:, m * 128 : (m + 1) * 128],
                    in_=wq_dt[t][:, m * 128 : (m + 1) * 128],
                )
        wv_sb = []
        for t in range(NT):
            wt = sg.tile([128, C], bf16, name=f"wv{t}", tag=f"wv{t}")
            nc.sync.dma_start(out=wt, in_=wv_dt[t])
            wv_sb.append(wt)
        for m in (2, 8, 9, 3, 10, 11):
            for t in range(NT):
                nc.sync.dma_start(
                    out=wq_sb[t][:, m * 128 : (m + 1) * 128],
                    in_=wq_dt[t][:, m * 128 : (m + 1) * 128],
                )
        wp_sb = []
        for t in range(NT):
            wt = sg.tile([128, C], bf16, name=f"wp{t}", tag=f"wp{t}")
            nc.sync.dma_start(out=wt, in_=wp_dt[t])
            wp_sb.append(wt)

        # vt tiles [keys, head, 64 v | 64 ones]; ones via gpsimd (idle engine)
        vt_sb = []
        for kt in range(KT):
            vt = work.tile([128, NH, 128], bf16, name=f"vt{kt}", tag=f"vt{kt}")
            nc.gpsimd.memset(vt[:, :, HD:128], 1.0)
            vt_sb.append(vt)

        eps_sb = sg.tile([8, 1], f32, name="eps_sb")
        nc.vector.memset(eps_sb, EPS)

        # ---- GroupNorm statistics ----
        allstats = sg.tile([128, 2 * NT], f32, name="allstats")
        for t in range(NT):
            bns = small.tile([128, 4, 6], f32, name=f"bns{t}", tag="bns")
            for q in range(4):
                nc.vector.bn_stats(
                    out=bns[:, q, :], in_=x_sb[t][:, q * 256 : (q + 1) * 256]
                )
            nc.vector.bn_aggr(out=allstats[:, 2 * t : 2 * t + 2], in_=bns)
            # E[x^2] = var + mean^2 into the odd column
            m2 = small.tile([128, 1], f32, name=f"m2_{t}", tag="m2")
            nc.vector.tensor_mul(
                m2, allstats[:, 2 * t : 2 * t + 1], allstats[:, 2 * t : 2 * t + 1]
            )
            nc.vector.tensor_add(
                allstats[:, 2 * t + 1 : 2 * t + 2],
                allstats[:, 2 * t + 1 : 2 * t + 2],
                m2,
            )

        # group aggregate: [8 local groups, 2*NT stats]
        grp_ps = psc.tile([8, 2 * NT], f32, name="grp_ps", tag="sc")
        nc.tensor.matmul(grp_ps, gmask_sb, allstats)
        grp_sb = sg.tile([8, 2 * NT], f32, name="grp_sb")
        nc.vector.tensor_copy(grp_sb, grp_ps)
        # var = E[x^2] - mean^2 ; rstd = 1/sqrt(var+eps)  (in cols 1::2)
        msq = sg.tile([8, NT], f32, name="msq")
        nc.vector.tensor_mul(msq, grp_sb[:, 0 : 2 * NT : 2], grp_sb[:, 0 : 2 * NT : 2])
        nc.vector.tensor_sub(
            grp_sb[:, 1 : 2 * NT : 2], grp_sb[:, 1 : 2 * NT : 2], msq
        )
        # The ONLY activation table this kernel ever loads is set 6
        # ('natural_log_exp_and_others': exp AND ln).  rstd is computed as
        # exp(-0.5*ln(var+eps)) instead of sqrt+reciprocal, so the GN, the
        # probs exps and the denominator ln/exp all run on one set: a single
        # table load at kernel start, zero switches (the baseline paid 8
        # switches, ~1.3us each).  The load is hand-emitted (the auto pass is
        # disabled) and carries no data deps, so it executes before every
        # activation — safe precisely because it is the only set.
        nc.scalar.add_instruction(
            mybir.InstLoadActFuncSet(
                name=nc.get_next_instruction_name(), ins=[], outs=[],
                act_func_set_id=6,
            )
        )
        lnv = small.tile([8, NT], f32, name="lnv", tag="lnv")
        nc.scalar.activation(
            out=lnv,
            in_=grp_sb[:, 1 : 2 * NT : 2],
            func=AF.Ln,
            bias=eps_sb,
            scale=1.0,
        )
        nc.scalar.activation(
            out=grp_sb[:, 1 : 2 * NT : 2], in_=lnv, func=AF.Exp, scale=-0.5
        )

        # broadcast group stats back to channels: [128, 2*NT]
        chan_ps = psc.tile([128, 2 * NT], f32, name="chan_ps", tag="sc")
        nc.tensor.matmul(chan_ps, gmaskT_sb, grp_sb)
        chan_sb = sg.tile([128, 2 * NT], f32, name="chan_sb")
        nc.vector.tensor_copy(chan_sb, chan_ps)

        # PE clock warm-up: the tensor engine p-state ramps with sustained
        # use (max clock only after ~3us continuous).  While the GN-apply
        # runs on DVE/GPSIMD the PE would idle and start the QKV phase at
        # half clock; stream a few throwaway matmuls on already-loaded
        # weight tiles to carry the busy streak into the QKV chains.
        for w in range(6):
            warm = psc.tile([128, 512], f32, name=f"warm{w}", tag="sc")
            nc.tensor.matmul(
                warm, wq_sb[0][:, 0:128], wq_sb[0][:, 0:512]
            )

        # A = rstd * gn_w ; Bc = gn_b - mean * A   (per channel, per tile col)
        A_sb = sg.tile([128, NT], f32, name="A_sb")
        nc.vector.tensor_mul(A_sb, chan_sb[:, 1 : 2 * NT : 2], gnw_sb)
        B_sb = sg.tile([128, NT], f32, name="B_sb")
        nc.vector.tensor_mul(B_sb, chan_sb[:, 0 : 2 * NT : 2], A_sb)
        nc.vector.tensor_sub(B_sb, gnb_sb, B_sb)

        # ---- GN apply: split across DVE and GPSIMD so the four tiles
        # finish ~2x sooner (the first QKV chains wait on xn) ----
        xn_sb = []
        for t in range(NT):
            xn = work.tile([128, N], bf16, name=f"xn{t}", tag=f"xn{t}")
            eng = nc.vector if t % 2 == 0 else nc.gpsimd
            eng.tensor_scalar(
                out=xn,
                in0=x_sb[t],
                scalar1=A_sb[:, t : t + 1],
                scalar2=B_sb[:, t : t + 1],
                op0=OP.mult,
                op1=OP.add,
            )
            xn_sb.append(xn)

        # ---- work units (each: one 4-step PSUM chain in an sc/cx slot) ----
        qkv_sb = [None] * MT

        def qkv_half(mt, hlf, pool, tag):
            if qkv_sb[mt] is None:
                qkv_sb[mt] = work.tile(
                    [128, N], bf16, name=f"qkv{mt}", tag=f"qkv{mt}"
                )
            qp = pool.tile([128, 512], f32, name=f"qp{mt}_{hlf}", tag=tag)
            for kc in range(NT):
                nc.tensor.matmul(
                    qp,
                    wq_sb[kc][:, mt * 128 : (mt + 1) * 128],
                    xn_sb[kc][:, hlf * 512 : (hlf + 1) * 512],
                    start=(kc == 0),
                    stop=(kc == NT - 1),
                )
            nc.vector.tensor_scalar_add(
                qkv_sb[mt][:, hlf * 512 : (hlf + 1) * 512],
                qp,
                bqkv_sb[:, mt : mt + 1],
            )

        def vt_unit(kt, pool, tag):
            vp = pool.tile([128, NH, HD], f32, name=f"vp{kt}", tag=tag)
            for kc in range(NT):
                nc.tensor.matmul(
                    vp,
                    xn_sb[kc][:, kt * 128 : (kt + 1) * 128],
                    wv_sb[kc],
                    start=(kc == 0),
                    stop=(kc == NT - 1),
                )
            nc.vector.tensor_copy(vt_sb[kt][:, :, 0:HD], vp)

        # ---- attention pair with injected filler units ----
        hatt_sb = []
        for t in range(NT):
            ht = work.tile([128, N], bf16, name=f"hatt{t}", tag=f"hatt{t}")
            hatt_sb.append(ht)

        def attn_pair(j, units, prev_norm=None):
            """units: list of callables unit(pool, tag) emitting one chain.
            prev_norm: previous pair's deferred normalize — emitted after
            this pair's first scores so the Ln never stalls the in-order ACT
            queue waiting on the previous pair's last context matmul."""
            h0, h1 = 2 * j, 2 * j + 1
            cx = {}
            for h in (h0, h1):
                cx[h] = pcx.tile([128, N], f32, name=f"cx{h}", tag="cx")
            pbs = {}
            units = list(units)
            # inject points: pair head (kt=0,1) and tail (kt=8,9) by default
            # (mid-pair chains delay the score matmuls the ACT waits on).
            # Pair 0's vT units are consumed by this very pair's context
            # matmuls, so they must land just-in-time mid-pair instead:
            # vT(kt) at loop position kt-2, two steps before C(kt) needs it.
            if j == 0:
                inject_at = {0: 1, 1: 1, 4: 1, 5: 1, 6: 1, 7: 1}
            else:
                inject_at = {0: 1, 1: 1, KT: 2, KT + 1: 2}

            def emit_sc(kt):
                sc = {}
                for h in (h0, h1):
                    sc[h] = psc.tile([128, N], f32, name=f"sc{h}_{kt}", tag="sc")
                for h in (h0, h1):
                    lhsT = qkv_sb[4 + h][:, kt * 128 : (kt + 1) * 128]
                    for hlf in range(2):
                        nc.tensor.matmul(
                            sc[h][:, hlf * 512 : (hlf + 1) * 512],
                            lhsT,
                            qkv_sb[h // 2][:, hlf * 512 : (hlf + 1) * 512],
                        )
                    pb = pb_pool.tile([128, N], bf16, name=f"pb{h}_{kt}", tag="pb")
                    nc.scalar.activation(
                        out=pb, in_=sc[h], func=AF.Exp, scale=1.0 / 8.0
                    )
                    pbs[(h, kt)] = pb

            def emit_cx(kt):
                for h in (h0, h1):
                    for hlf in range(2):
                        nc.tensor.matmul(
                            cx[h][:, hlf * 512 : (hlf + 1) * 512],
                            vt_sb[kt][:, h, :],
                            pbs[(h, kt)][:, hlf * 512 : (hlf + 1) * 512],
                            start=(kt == 0),
                            stop=(kt == KT - 1),
                        )

            for kt in range(KT + LAG):
                if kt < KT:
                    emit_sc(kt)
                if kt == 1 and prev_norm is not None:
                    prev_norm()
                for _ in range(inject_at.get(kt, 0)):
                    if units:
                        units.pop(0)(psc, "sc")
                if kt >= LAG:
                    emit_cx(kt - LAG)
            while units:
                units.pop(0)(psc, "sc")

            # rows 64-127 of cx hold the softmax denominator per query
            # (vt ones block).  1/d = exp(-ln(d)) on ACT: 1 elem/lane/cycle,
            # vs ~6.3 cycles/elem for the DVE's iterative reciprocal, and the
            # combined LUT set means no table switches.  Returned as a
            # deferred closure; the caller emits it inside the NEXT pair.
            def normalize():
                for h in (h0, h1):
                    lnd = small.tile(
                        [HD, N], f32, name=f"lnd{h}", tag="lnd", bufs=2
                    )
                    nc.scalar.activation(out=lnd, in_=cx[h][HD:128, :], func=AF.Ln)
                    rsb = small.tile(
                        [HD, N], f32, name=f"rsb{h}", tag="rsb", bufs=2
                    )
                    nc.scalar.activation(out=rsb, in_=lnd, func=AF.Exp, scale=-1.0)
                    po = (h % 2) * HD
                    nc.vector.tensor_mul(
                        hatt_sb[j][po : po + HD, :], cx[h][0:HD, :], rsb
                    )

            return normalize

        # ---- PE program: pre-phase then pipelined pairs ----
        # pre: QKV m-tiles for pairs 0 and 1, then the first two vT chains
        for mt in (0, 4, 5, 1, 6, 7):
            qkv_half(mt, 0, psc, "sc")
            qkv_half(mt, 1, pcx, "cx")
        vt_unit(0, psc, "sc")
        vt_unit(1, pcx, "cx")

        def mk_vt(kt):
            return lambda pool, tag: vt_unit(kt, pool, tag)

        def mk_qkv(mt, hlf):
            return lambda pool, tag: qkv_half(mt, hlf, pool, tag)

        norm = attn_pair(0, [mk_vt(kt) for kt in range(2, KT)])
        norm = attn_pair(
            1, [mk_qkv(mt, hlf) for mt in (2, 8, 9) for hlf in range(2)], norm
        )
        norm = attn_pair(
            2, [mk_qkv(mt, hlf) for mt in (3, 10, 11) for hlf in range(2)], norm
        )
        norm = attn_pair(3, [], norm)
        norm()

        # ---- proj + bias + residual (half-N accumulators) ----
        for mt in range(NT):
            ot = outp.tile([128, N], f32, name=f"ot{mt}", tag="ot")
            for hlf in range(2):
                ppool, ptag = (psc, "sc") if hlf == 0 else (pcx, "cx")
                pp = ppool.tile([128, 512], f32, name=f"pp{mt}_{hlf}", tag=ptag)
                for kc in range(NT):
                    nc.tensor.matmul(
                        pp,
                        wp_sb[kc][:, mt * 128 : (mt + 1) * 128],
                        hatt_sb[kc][:, hlf * 512 : (hlf + 1) * 512],
                        start=(kc == 0),
                        stop=(kc == NT - 1),
                    )
                nc.vector.scalar_tensor_tensor(
                    out=ot[:, hlf * 512 : (hlf + 1) * 512],
                    in0=pp,
                    scalar=bproj_sb[:, mt : mt + 1],
                    in1=x_sb[mt][:, hlf * 512 : (hlf + 1) * 512],
                    op0=OP.add,
                    op1=OP.add,
                )
                # fine-granular out DMAs: a single 512KB transfer would sit
                # on one queue (~25us); 64KB chunks fan out across queues
                # and start as soon as each half is drained.
                for q in range(4):
                    lo = hlf * 512 + q * 128
                    nc.sync.dma_start(
                        out=out_dt[mt][:, lo : lo + 128],
                        in_=ot[:, lo : lo + 128],
                    )

    nc.compile()
    return nc


def _get_nc():
    if "nc" not in _CACHE:
        _CACHE["nc"] = _build_program()
    return _CACHE["nc"]


def _host_inputs(x, gn_w, gn_b, qkv_w, qkv_b, proj_w, proj_b):
    f32 = np.float32
    x = np.asarray(x, dtype=f32).reshape(B, C, N)
    gn_w = np.asarray(gn_w, dtype=f32)
    gn_b = np.asarray(gn_b, dtype=f32)
    qkv_w = np.asarray(qkv_w, dtype=f32)
    qkv_b = np.asarray(qkv_b, dtype=f32)
    proj_w = np.asarray(proj_w, dtype=f32)
    proj_b = np.asarray(proj_b, dtype=f32)

    # device QKV weight layout: [W_q (512 cols) | K-padded (1024 cols: per
    # head h a 128-col block, k_h placed at rows (h%2)*64, zeros elsewhere)]
    import ml_dtypes

    bf16 = ml_dtypes.bfloat16
    wq_T = qkv_w[0:512].T                      # [C, 512]
    wk_T = qkv_w[512:1024].T                   # [C, 512]
    wv_T = np.ascontiguousarray(qkv_w[1024:1536].T.astype(bf16))  # [C, 512]
    kpad = np.zeros((512, 1024), np.float32)
    for h in range(8):
        po = (h % 2) * 64
        kpad[:, h * 128 + po : h * 128 + po + 64] = wk_T[:, h * 64 : (h + 1) * 64]
    wqkvT = np.ascontiguousarray(
        np.concatenate([wq_T, kpad], axis=1).astype(bf16)
    )
    # Q bias kept; K bias dropped (softmax-invariant: contributes only
    # per-query constants to the scores); V bias folded into the proj bias
    # (softmax rows sum to one, so attn(v + b_v) = attn(v) + b_v).
    bqkv_flat = np.concatenate([qkv_b[0:512], np.zeros(1024, f32)])
    bqkv = np.ascontiguousarray(bqkv_flat.reshape(MT, 128).T)
    wprojT = np.ascontiguousarray(proj_w.T.astype(bf16))
    bproj_folded = proj_b + proj_w @ qkv_b[1024:1536]
    bproj = np.ascontiguousarray(bproj_folded.astype(f32).reshape(NT, 128).T)
    gnw = np.ascontiguousarray(gn_w.reshape(NT, 128).T)
    gnb = np.ascontiguousarray(gn_b.reshape(NT, 128).T)

    p = np.arange(128)
    gmask = np.zeros((128, 8), f32)
    gmask[p, p // 16] = 1.0 / 16.0
    gmaskT = np.ascontiguousarray(
        (np.arange(128)[:, None] // 16 == np.arange(8)[None, :]).astype(f32).T
    )

    common = dict(
        wqkvT=wqkvT, wvT=wv_T, wprojT=wprojT, bqkv=bqkv, bproj=bproj,
        gnw=gnw, gnb=gnb, gmask=gmask, gmaskT=gmaskT,
    )
    return [dict(common, x=np.ascontiguousarray(x[b])) for b in range(B)]


def _run(in_maps, trace=False, **kw):
    from concourse.bass_utils import run_bass_kernel_spmd

    nc = _get_nc()
    return run_bass_kernel_spmd(nc, in_maps, list(range(NCORES)), trace=trace, **kw)


def kernel(x, gn_w, gn_b, qkv_w, qkv_b, proj_w, proj_b):
    in_maps = _host_inputs(x, gn_w, gn_b, qkv_w, qkv_b, proj_w, proj_b)
    res = _run(in_maps)
    out = np.stack([res.results[b]["out"] for b in range(B)])
    return out.reshape(B, C, HH, WW).astype(np.float32)


# revision 41
# speedup vs baseline: 1.1110x; 1.1110x over previous
"""AttentionBlock (GroupNorm -> QKV -> 8-head attention -> proj -> residual)
as a Bass/Tile kernel for Trainium2, data-parallel over batch on 8 cores.

Self-contained: hardcodes shapes B=8, C=512, H=W=32 (N=1024), heads=8, d=64,
groups=32.  Each core processes one batch element; all params replicated.

Key structure (v2 — globally software-pipelined):
  x [C, N] channel-major -> 4 SBUF tiles [128, 1024].
  GroupNorm: per-channel mean/var via bn_stats/bn_aggr, cross-partition group
  aggregation + broadcast via two tiny mask matmuls on the PE.
  QKV: only Q (m-tiles 0-3) and K-padded (m-tiles 4-11; head h occupies
  rows (h%2)*64 of tile 4+h, other rows zero so the K=128 contraction is
  head-exclusive).  V never materializes channel-major: vT[keys, 8*64] is
  computed directly as xn^T @ wvT (4-step chains per key tile), drained into
  bf16 vt tiles [128, 8, 128] whose odd 64-col halves are pre-memset to 1.0
  (gpsimd) — the ones block makes the context matmul broadcast the softmax
  denominator into output rows 64-127 for free.
  Biases: Q bias kept; K bias dropped (exactly softmax-invariant: it only
  adds per-query constants to scores); V bias folded into the proj bias on
  the host (proj_b + proj_w @ b_v, valid because softmax rows sum to 1).
  Attention per head pair (transposed orientation, no max-subtraction):
  scoresT = K^T Q on the PE, exp on ACT (scale=1/8) -> bf16 probs,
  contextT accumulated as vt^T @ probsT.  1/denominator via DVE reciprocal
  (NOT Ln/Exp on ACT — saves ~27us of ACT incl. table switches), multiply on
  DVE -> h_attT tiles.  proj: wprojT.T @ h_attT + bias' + x -> out.

Scheduling: the exp stream on ACT (64 x [128,1024] tiles ~ 67us) is the
critical resource; it is started as early as possible and kept fed.  PE
program order: GN mms -> QKV m-tiles for pairs 0-1 -> vT(kt=0,1) -> pair 0
(injecting vT(2..7) chains into its stream) -> pair 1 (injecting pair 2's
QKV chains) -> pair 2 (injecting pair 3's) -> pair 3 -> proj.  Injected
chains reuse the score-slot PSUM banks (tag sharing) at pair head/tail
where the exp pipeline covers them.  The Exp LUT is preloaded by a dummy
activation during the QKV phase so the first real exp pays no table load.
PSUM: scores 2 slots x 2 banks, context 2 slots x 2 banks = 8 banks.

Matmul inputs are float32r (1 cycle/row for moving free >= 256; fp32 would
be 4) except probs/vt which are bf16.  f32r operands must be *produced* as
f32r, so every tile feeding a matmul is allocated f32r.
"""

import sys

sys.path.insert(0, "/opt/trn_rl_repo")

import numpy as np

B, C, HH, WW = 8, 512, 32, 32
N = HH * WW          # 1024
NH, HD = 8, 64       # heads, head dim
NG = 32              # groupnorm groups
EPS = 1e-5
NT = C // 128        # 4 channel tiles
MT = 12              # qkv m-tiles: Q 0-3 | K-padded 4-11
KT = N // 128        # 8 key tiles
NCORES = 8
LAG = 2              # context matmuls run LAG k-tiles behind scores/exp
# qkv m-tiles stored on the host in first-use order so the weight DMAs are
# two big contiguous chunks per channel tile
MPERM = (0, 4, 5, 1, 6, 7, 2, 8, 9, 3, 10, 11)
MPOS = {m: i for i, m in enumerate(MPERM)}

_CACHE: dict = {}


def _build_program():
    import concourse.bacc as bacc
    import concourse.tile as tile
    from concourse import mybir

    f32 = mybir.dt.float32
    f32r = mybir.dt.float32r
    bf16 = mybir.dt.bfloat16
    AF = mybir.ActivationFunctionType
    OP = mybir.AluOpType

    nc = bacc.Bacc("TRN2", target_bir_lowering=False, debug=False)
    # We place activation-table loads by hand (sqrt set before the GN sqrt,
    # then the combined exp+ln set once for the whole attention phase).  The
    # automatic pass does not track hand-placed loads and would re-insert a
    # single-function set load at every exp<->ln transition (8 switches,
    # ~1.3us each), so disable it for this program.
    nc.insert_act_table_loads = lambda: None

    x_d = nc.dram_tensor("x", [C, N], f32, kind="ExternalInput").ap()
    # weights in bf16: halves the weight DMA traffic and doubles the PE
    # ldweights rate (bf16 loads 1 row/cycle vs ~2.5 for f32r); the moving
    # operands stay f32r so matmuls still stream at 1 cycle/row.
    wqkv_d = nc.dram_tensor("wqkvT", [C, MT * 128], bf16, kind="ExternalInput").ap()
    wv_d = nc.dram_tensor("wvT", [C, C], bf16, kind="ExternalInput").ap()
    wproj_d = nc.dram_tensor("wprojT", [C, C], bf16, kind="ExternalInput").ap()
    bqkv_d = nc.dram_tensor("bqkv", [128, MT], f32, kind="ExternalInput").ap()
    bproj_d = nc.dram_tensor("bproj", [128, NT], f32, kind="ExternalInput").ap()
    gnw_d = nc.dram_tensor("gnw", [128, NT], f32, kind="ExternalInput").ap()
    gnb_d = nc.dram_tensor("gnb", [128, NT], f32, kind="ExternalInput").ap()
    gmask_d = nc.dram_tensor("gmask", [128, 8], f32, kind="ExternalInput").ap()
    gmaskT_d = nc.dram_tensor("gmaskT", [8, 128], f32, kind="ExternalInput").ap()
    out_d = nc.dram_tensor("out", [C, N], f32, kind="ExternalOutput").ap()

    x_dt = x_d.rearrange("(t p) n -> t p n", p=128)
    out_dt = out_d.rearrange("(t p) n -> t p n", p=128)
    wq_dt = wqkv_d.rearrange("(t p) m -> t p m", p=128)
    wv_dt = wv_d.rearrange("(t p) m -> t p m", p=128)
    wp_dt = wproj_d.rearrange("(t p) m -> t p m", p=128)

    from contextlib import ExitStack

    with tile.TileContext(nc) as tc, ExitStack() as ctx:
        sg = ctx.enter_context(tc.tile_pool(name="sg", bufs=1))
        work = ctx.enter_context(tc.tile_pool(name="work", bufs=1))
        pb_pool = ctx.enter_context(tc.tile_pool(name="pbp", bufs=8))
        small = ctx.enter_context(tc.tile_pool(name="small", bufs=4))
        outp = ctx.enter_context(tc.tile_pool(name="outp", bufs=2))
        # PSUM budget (8 banks): "sc" slots 2x2 banks (score tiles; shared by
        # the QKV/vT/proj half-accumulator chains and the GN matmuls via tag
        # reuse), "cx" slots 2x2 banks (context accumulators of a pair; also
        # chain slots outside attention).
        psc = ctx.enter_context(tc.tile_pool(name="psc", bufs=2, space="PSUM"))
        pcx = ctx.enter_context(tc.tile_pool(name="pcx", bufs=2, space="PSUM"))

        # ---- x first (critical path for GN): quarter-granular DMAs so
        # bn_stats can start on each quarter as it lands ----
        x_sb = []
        for t in range(NT):
            xt = work.tile([128, N], f32, name=f"x{t}", tag=f"x{t}")
            for q in range(4):
                nc.sync.dma_start(
                    out=xt[:, q * 256 : (q + 1) * 256],
                    in_=x_dt[t][:, q * 256 : (q + 1) * 256],
                )
            x_sb.append(xt)

        # ---- small constants (biases needed at first drains) ----
        bqkv_sb = sg.tile([128, MT], f32, name="bqkv_sb")
        nc.sync.dma_start(out=bqkv_sb, in_=bqkv_d)
        bproj_sb = sg.tile([128, NT], f32, name="bproj_sb")
        nc.sync.dma_start(out=bproj_sb, in_=bproj_d)
        gnw_sb = sg.tile([128, NT], f32, name="gnw_sb")
        nc.sync.dma_start(out=gnw_sb, in_=gnw_d)
        gnb_sb = sg.tile([128, NT], f32, name="gnb_sb")
        nc.sync.dma_start(out=gnb_sb, in_=gnb_d)
        gmask_sb = sg.tile([128, 8], f32, name="gmask_sb")
        nc.sync.dma_start(out=gmask_sb, in_=gmask_d)
        gmaskT_sb = sg.tile([8, 128], f32, name="gmaskT_sb")
        nc.sync.dma_start(out=gmaskT_sb, in_=gmaskT_d)

        # ---- weights, ordered by first use.  The host stores the qkv
        # m-tiles pre-permuted into first-use order (MPERM), so each DMA is
        # one big contiguous chunk (bf16 needs >= 512B per partition row to
        # avoid the 2x small-element DMA penalty; 128-col bf16 blocks would
        # be 256B and trickle in far too slowly). ----
        wq_sb = []
        for t in range(NT):
            wt = sg.tile([128, MT * 128], bf16, name=f"wq{t}", tag=f"wq{t}")
            wq_sb.append(wt)
        for t in range(NT):  # pre-phase m-tiles (first 6 in MPERM order)
            nc.sync.dma_start(out=wq_sb[t][:, 0:768], in_=wq_dt[t][:, 0:768])
        wv_sb = []
        for t in range(NT):
            wt = sg.tile([128, C], bf16, name=f"wv{t}", tag=f"wv{t}")
            nc.sync.dma_start(out=wt, in_=wv_dt[t])
            wv_sb.append(wt)
        for t in range(NT):  # remaining m-tiles
            nc.sync.dma_start(
                out=wq_sb[t][:, 768:1536], in_=wq_dt[t][:, 768:1536]
            )
        wp_sb = []
        for t in range(NT):
            wt = sg.tile([128, C], bf16, name=f"wp{t}", tag=f"wp{t}")
            nc.sync.dma_start(out=wt, in_=wp_dt[t])
            wp_sb.append(wt)

        # vt tiles [keys, head, 64 v | 64 ones]; ones via gpsimd (idle engine)
        vt_sb = []
        for kt in range(KT):
            vt = work.tile([128, NH, 128], bf16, name=f"vt{kt}", tag=f"vt{kt}")
            nc.gpsimd.memset(vt[:, :, HD:128], 1.0)
            vt_sb.append(vt)

        eps_sb = sg.tile([8, 1], f32, name="eps_sb")
        nc.vector.memset(eps_sb, EPS)

        # ---- GroupNorm statistics ----
        allstats = sg.tile([128, 2 * NT], f32, name="allstats")
        for t in range(NT):
            bns = small.tile([128, 4, 6], f32, name=f"bns{t}", tag="bns")
            for q in range(4):
                nc.vector.bn_stats(
                    out=bns[:, q, :], in_=x_sb[t][:, q * 256 : (q + 1) * 256]
                )
            nc.vector.bn_aggr(out=allstats[:, 2 * t : 2 * t + 2], in_=bns)
            # E[x^2] = var + mean^2 into the odd column
            m2 = small.tile([128, 1], f32, name=f"m2_{t}", tag="m2")
            nc.vector.tensor_mul(
                m2, allstats[:, 2 * t : 2 * t + 1], allstats[:, 2 * t : 2 * t + 1]
            )
            nc.vector.tensor_add(
                allstats[:, 2 * t + 1 : 2 * t + 2],
                allstats[:, 2 * t + 1 : 2 * t + 2],
                m2,
            )

        # group aggregate: [8 local groups, 2*NT stats]
        grp_ps = psc.tile([8, 2 * NT], f32, name="grp_ps", tag="sc")
        nc.tensor.matmul(grp_ps, gmask_sb, allstats)
        grp_sb = sg.tile([8, 2 * NT], f32, name="grp_sb")
        nc.vector.tensor_copy(grp_sb, grp_ps)
        # var = E[x^2] - mean^2 ; rstd = 1/sqrt(var+eps)  (in cols 1::2)
        msq = sg.tile([8, NT], f32, name="msq")
        nc.vector.tensor_mul(msq, grp_sb[:, 0 : 2 * NT : 2], grp_sb[:, 0 : 2 * NT : 2])
        nc.vector.tensor_sub(
            grp_sb[:, 1 : 2 * NT : 2], grp_sb[:, 1 : 2 * NT : 2], msq
        )
        # The ONLY activation table this kernel ever loads is set 6
        # ('natural_log_exp_and_others': exp AND ln).  rstd is computed as
        # exp(-0.5*ln(var+eps)) instead of sqrt+reciprocal, so the GN, the
        # probs exps and the denominator ln/exp all run on one set: a single
        # table load at kernel start, zero switches (the baseline paid 8
        # switches, ~1.3us each).  The load is hand-emitted (the auto pass is
        # disabled) and carries no data deps, so it executes before every
        # activation — safe precisely because it is the only set.
        nc.scalar.add_instruction(
            mybir.InstLoadActFuncSet(
                name=nc.get_next_instruction_name(), ins=[], outs=[],
                act_func_set_id=6,
            )
        )
        lnv = small.tile([8, NT], f32, name="lnv", tag="lnv")
        nc.scalar.activation(
            out=lnv,
            in_=grp_sb[:, 1 : 2 * NT : 2],
            func=AF.Ln,
            bias=eps_sb,
            scale=1.0,
        )
        nc.scalar.activation(
            out=grp_sb[:, 1 : 2 * NT : 2], in_=lnv, func=AF.Exp, scale=-0.5
        )

        # broadcast group stats back to channels: [128, 2*NT]
        chan_ps = psc.tile([128, 2 * NT], f32, name="chan_ps", tag="sc")
        nc.tensor.matmul(chan_ps, gmaskT_sb, grp_sb)
        chan_sb = sg.tile([128, 2 * NT], f32, name="chan_sb")
        nc.vector.tensor_copy(chan_sb, chan_ps)

        # PE clock warm-up: the tensor engine p-state ramps with sustained
        # use (max clock only after ~3us continuous).  While the GN-apply
        # runs on DVE/GPSIMD the PE would idle and start the QKV phase at
        # half clock; stream a few throwaway matmuls on already-loaded
        # weight tiles to carry the busy streak into the QKV chains.
        for w in range(6):
            warm = psc.tile([128, 512], f32, name=f"warm{w}", tag="sc")
            nc.tensor.matmul(
                warm, wq_sb[0][:, 0:128], wq_sb[0][:, 0:512]
            )

        # A = rstd * gn_w ; Bc = gn_b - mean * A   (per channel, per tile col)
        A_sb = sg.tile([128, NT], f32, name="A_sb")
        nc.vector.tensor_mul(A_sb, chan_sb[:, 1 : 2 * NT : 2], gnw_sb)
        B_sb = sg.tile([128, NT], f32, name="B_sb")
        nc.vector.tensor_mul(B_sb, chan_sb[:, 0 : 2 * NT : 2], A_sb)
        nc.vector.tensor_sub(B_sb, gnb_sb, B_sb)

        # ---- GN apply: split across DVE and GPSIMD so the four tiles
        # finish ~2x sooner (the first QKV chains wait on xn) ----
        xn_sb = []
        for t in range(NT):
            xn = work.tile([128, N], bf16, name=f"xn{t}", tag=f"xn{t}")
            eng = nc.vector if t % 2 == 0 else nc.gpsimd
            eng.tensor_scalar(
                out=xn,
                in0=x_sb[t],
                scalar1=A_sb[:, t : t + 1],
                scalar2=B_sb[:, t : t + 1],
                op0=OP.mult,
                op1=OP.add,
            )
            xn_sb.append(xn)

        # ---- work units (each: one 4-step PSUM chain in an sc/cx slot) ----
        qkv_sb = [None] * MT

        def qkv_half(mt, hlf, pool, tag):
            if qkv_sb[mt] is None:
                qkv_sb[mt] = work.tile(
                    [128, N], bf16, name=f"qkv{mt}", tag=f"qkv{mt}"
                )
            qp = pool.tile([128, 512], f32, name=f"qp{mt}_{hlf}", tag=tag)
            mp = MPOS[mt]
            for kc in range(NT):
                nc.tensor.matmul(
                    qp,
                    wq_sb[kc][:, mp * 128 : (mp + 1) * 128],
                    xn_sb[kc][:, hlf * 512 : (hlf + 1) * 512],
                    start=(kc == 0),
                    stop=(kc == NT - 1),
                )
            nc.vector.tensor_scalar_add(
                qkv_sb[mt][:, hlf * 512 : (hlf + 1) * 512],
                qp,
                bqkv_sb[:, mt : mt + 1],
            )

        def vt_unit(kt, pool, tag):
            vp = pool.tile([128, NH, HD], f32, name=f"vp{kt}", tag=tag)
            for kc in range(NT):
                nc.tensor.matmul(
                    vp,
                    xn_sb[kc][:, kt * 128 : (kt + 1) * 128],
                    wv_sb[kc],
                    start=(kc == 0),
                    stop=(kc == NT - 1),
                )
            nc.vector.tensor_copy(vt_sb[kt][:, :, 0:HD], vp)

        # ---- attention pair with injected filler units ----
        hatt_sb = []
        for t in range(NT):
            ht = work.tile([128, N], bf16, name=f"hatt{t}", tag=f"hatt{t}")
            hatt_sb.append(ht)

        def attn_pair(j, units, prev_norm=None):
            """units: list of callables unit(pool, tag) emitting one chain.
            prev_norm: previous pair's deferred normalize — emitted after
            this pair's first scores so the Ln never stalls the in-order ACT
            queue waiting on the previous pair's last context matmul."""
            h0, h1 = 2 * j, 2 * j + 1
            cx = {}
            for h in (h0, h1):
                cx[h] = pcx.tile([128, N], f32, name=f"cx{h}", tag="cx")
            pbs = {}
            units = list(units)
            # inject points: pair head (kt=0,1) and tail (kt=8,9) by default
            # (mid-pair chains delay the score matmuls the ACT waits on).
            # Pair 0's vT units are consumed by this very pair's context
            # matmuls, so they must land just-in-time mid-pair instead:
            # vT(kt) at loop position kt-2, two steps before C(kt) needs it.
            if j == 0:
                inject_at = {0: 1, 1: 1, 4: 1, 5: 1, 6: 1, 7: 1}
            else:
                inject_at = {0: 1, 1: 1, KT: 2, KT + 1: 2}

            def emit_sc(kt):
                sc = {}
                for h in (h0, h1):
                    sc[h] = psc.tile([128, N], f32, name=f"sc{h}_{kt}", tag="sc")
                for h in (h0, h1):
                    lhsT = qkv_sb[4 + h][:, kt * 128 : (kt + 1) * 128]
                    for hlf in range(2):
                        nc.tensor.matmul(
                            sc[h][:, hlf * 512 : (hlf + 1) * 512],
                            lhsT,
                            qkv_sb[h // 2][:, hlf * 512 : (hlf + 1) * 512],
                        )
                    pb = pb_pool.tile([128, N], bf16, name=f"pb{h}_{kt}", tag="pb")
                    nc.scalar.activation(
                        out=pb, in_=sc[h], func=AF.Exp, scale=1.0 / 8.0
                    )
                    pbs[(h, kt)] = pb

            def emit_cx(kt):
                for h in (h0, h1):
                    for hlf in range(2):
                        nc.tensor.matmul(
                            cx[h][:, hlf * 512 : (hlf + 1) * 512],
                            vt_sb[kt][:, h, :],
                            pbs[(h, kt)][:, hlf * 512 : (hlf + 1) * 512],
                            start=(kt == 0),
                            stop=(kt == KT - 1),
                        )

            for kt in range(KT + LAG):
                if kt < KT:
                    emit_sc(kt)
                if kt == 1 and prev_norm is not None:
                    prev_norm()
                for _ in range(inject_at.get(kt, 0)):
                    if units:
                        units.pop(0)(psc, "sc")
                if kt >= LAG:
                    emit_cx(kt - LAG)
            while units:
                units.pop(0)(psc, "sc")

            # rows 64-127 of cx hold the softmax denominator per query
            # (vt ones block).  1/d = exp(-ln(d)) on ACT: 1 elem/lane/cycle,
            # vs ~6.3 cycles/elem for the DVE's iterative reciprocal, and the
            # combined LUT set means no table switches.  Returned as a
            # deferred closure; the caller emits it inside the NEXT pair.
            def normalize():
                for h in (h0, h1):
                    lnd = small.tile(
                        [HD, N], f32, name=f"lnd{h}", tag="lnd", bufs=2
                    )
                    nc.scalar.activation(out=lnd, in_=cx[h][HD:128, :], func=AF.Ln)
                    rsb = small.tile(
                        [HD, N], f32, name=f"rsb{h}", tag="rsb", bufs=2
                    )
                    nc.scalar.activation(out=rsb, in_=lnd, func=AF.Exp, scale=-1.0)
                    po = (h % 2) * HD
                    nc.vector.tensor_mul(
                        hatt_sb[j][po : po + HD, :], cx[h][0:HD, :], rsb
                    )

            return normalize

        # ---- PE program: pre-phase then pipelined pairs ----
        # pre: QKV m-tiles for pairs 0 and 1, then the first two vT chains
        for mt in (0, 4, 5, 1, 6, 7):
            qkv_half(mt, 0, psc, "sc")
            qkv_half(mt, 1, pcx, "cx")
        vt_unit(0, psc, "sc")
        vt_unit(1, pcx, "cx")

        def mk_vt(kt):
            return lambda pool, tag: vt_unit(kt, pool, tag)

        def mk_qkv(mt, hlf):
            return lambda pool, tag: qkv_half(mt, hlf, pool, tag)

        norm = attn_pair(0, [mk_vt(kt) for kt in range(2, KT)])
        norm = attn_pair(
            1, [mk_qkv(mt, hlf) for mt in (2, 8, 9) for hlf in range(2)], norm
        )
        norm = attn_pair(
            2, [mk_qkv(mt, hlf) for mt in (3, 10, 11) for hlf in range(2)], norm
        )
        norm = attn_pair(3, [], norm)
        norm()

        # ---- proj + bias + residual (half-N accumulators) ----
        for mt in range(NT):
            ot = outp.tile([128, N], f32, name=f"ot{mt}", tag="ot")
            for hlf in range(2):
                ppool, ptag = (psc, "sc") if hlf == 0 else (pcx, "cx")
                pp = ppool.tile([128, 512], f32, name=f"pp{mt}_{hlf}", tag=ptag)
                for kc in range(NT):
                    nc.tensor.matmul(
                        pp,
                        wp_sb[kc][:, mt * 128 : (mt + 1) * 128],
                        hatt_sb[kc][:, hlf * 512 : (hlf + 1) * 512],
                        start=(kc == 0),
                        stop=(kc == NT - 1),
                    )
                nc.vector.scalar_tensor_tensor(
                    out=ot[:, hlf * 512 : (hlf + 1) * 512],
                    in0=pp,
                    scalar=bproj_sb[:, mt : mt + 1],
                    in1=x_sb[mt][:, hlf * 512 : (hlf + 1) * 512],
                    op0=OP.add,
                    op1=OP.add,
                )
                # fine-granular out DMAs: a single 512KB transfer would sit
                # on one queue (~25us); 64KB chunks fan out across queues
                # and start as soon as each half is drained.
                for q in range(4):
                    lo = hlf * 512 + q * 128
                    nc.sync.dma_start(
                        out=out_dt[mt][:, lo : lo + 128],
                        in_=ot[:, lo : lo + 128],
                    )

    nc.compile()
    return nc


def _get_nc():
    if "nc" not in _CACHE:
        _CACHE["nc"] = _build_program()
    return _CACHE["nc"]


def _host_inputs(x, gn_w, gn_b, qkv_w, qkv_b, proj_w, proj_b):
    f32 = np.float32
    x = np.asarray(x, dtype=f32).reshape(B, C, N)
    gn_w = np.asarray(gn_w, dtype=f32)
    gn_b = np.asarray(gn_b, dtype=f32)
    qkv_w = np.asarray(qkv_w, dtype=f32)
    qkv_b = np.asarray(qkv_b, dtype=f32)
    proj_w = np.asarray(proj_w, dtype=f32)
    proj_b = np.asarray(proj_b, dtype=f32)

    # device QKV weight layout: [W_q (512 cols) | K-padded (1024 cols: per
    # head h a 128-col block, k_h placed at rows (h%2)*64, zeros elsewhere)]
    import ml_dtypes

    bf16 = ml_dtypes.bfloat16
    wq_T = qkv_w[0:512].T                      # [C, 512]
    wk_T = qkv_w[512:1024].T                   # [C, 512]
    wv_T = np.ascontiguousarray(qkv_w[1024:1536].T.astype(bf16))  # [C, 512]
    kpad = np.zeros((512, 1024), np.float32)
    for h in range(8):
        po = (h % 2) * 64
        kpad[:, h * 128 + po : h * 128 + po + 64] = wk_T[:, h * 64 : (h + 1) * 64]
    wqkv_full = np.concatenate([wq_T, kpad], axis=1)
    # permute m-tiles into first-use order (see MPERM in the kernel)
    wqkv_perm = np.concatenate(
        [wqkv_full[:, m * 128 : (m + 1) * 128] for m in MPERM], axis=1
    )
    wqkvT = np.ascontiguousarray(wqkv_perm.astype(bf16))
    # Q bias kept; K bias dropped (softmax-invariant: contributes only
    # per-query constants to the scores); V bias folded into the proj bias
    # (softmax rows sum to one, so attn(v + b_v) = attn(v) + b_v).
    bqkv_flat = np.concatenate([qkv_b[0:512], np.zeros(1024, f32)])
    bqkv = np.ascontiguousarray(bqkv_flat.reshape(MT, 128).T)
    wprojT = np.ascontiguousarray(proj_w.T.astype(bf16))
    bproj_folded = proj_b + proj_w @ qkv_b[1024:1536]
    bproj = np.ascontiguousarray(bproj_folded.astype(f32).reshape(NT, 128).T)
    gnw = np.ascontiguousarray(gn_w.reshape(NT, 128).T)
    gnb = np.ascontiguousarray(gn_b.reshape(NT, 128).T)

    p = np.arange(128)
    gmask = np.zeros((128, 8), f32)
    gmask[p, p // 16] = 1.0 / 16.0
    gmaskT = np.ascontiguousarray(
        (np.arange(128)[:, None] // 16 == np.arange(8)[None, :]).astype(f32).T
    )

    common = dict(
        wqkvT=wqkvT, wvT=wv_T, wprojT=wprojT, bqkv=bqkv, bproj=bproj,
        gnw=gnw, gnb=gnb, gmask=gmask, gmaskT=gmaskT,
    )
    return [dict(common, x=np.ascontiguousarray(x[b])) for b in range(B)]


def _run(in_maps, trace=False, **kw):
    from concourse.bass_utils import run_bass_kernel_spmd

    nc = _get_nc()
    return run_bass_kernel_spmd(nc, in_maps, list(range(NCORES)), trace=trace, **kw)


def kernel(x, gn_w, gn_b, qkv_w, qkv_b, proj_w, proj_b):
    in_maps = _host_inputs(x, gn_w, gn_b, qkv_w, qkv_b, proj_w, proj_b)
    res = _run(in_maps)
    out = np.stack([res.results[b]["out"] for b in range(B)])
    return out.reshape(B, C, HH, WW).astype(np.float32)


# revision 43
# speedup vs baseline: 1.1789x; 1.0611x over previous
"""AttentionBlock (GroupNorm -> QKV -> 8-head attention -> proj -> residual)
as a Bass/Tile kernel for Trainium2, data-parallel over batch on 8 cores.

Self-contained: hardcodes shapes B=8, C=512, H=W=32 (N=1024), heads=8, d=64,
groups=32.  Each core processes one batch element; all params replicated.

Key structure (v2 — globally software-pipelined):
  x [C, N] channel-major -> 4 SBUF tiles [128, 1024].
  GroupNorm: per-channel mean/var via bn_stats/bn_aggr, cross-partition group
  aggregation + broadcast via two tiny mask matmuls on the PE.
  QKV: only Q (m-tiles 0-3) and K-padded (m-tiles 4-11; head h occupies
  rows (h%2)*64 of tile 4+h, other rows zero so the K=128 contraction is
  head-exclusive).  V never materializes channel-major: vT[keys, 8*64] is
  computed directly as xn^T @ wvT (4-step chains per key tile), drained into
  bf16 vt tiles [128, 8, 128] whose odd 64-col halves are pre-memset to 1.0
  (gpsimd) — the ones block makes the context matmul broadcast the softmax
  denominator into output rows 64-127 for free.
  Biases: Q bias kept; K bias dropped (exactly softmax-invariant: it only
  adds per-query constants to scores); V bias folded into the proj bias on
  the host (proj_b + proj_w @ b_v, valid because softmax rows sum to 1).
  Attention per head pair (transposed orientation, no max-subtraction):
  scoresT = K^T Q on the PE, exp on ACT (scale=1/8) -> bf16 probs,
  contextT accumulated as vt^T @ probsT.  1/denominator via DVE reciprocal
  (NOT Ln/Exp on ACT — saves ~27us of ACT incl. table switches), multiply on
  DVE -> h_attT tiles.  proj: wprojT.T @ h_attT + bias' + x -> out.

Scheduling: the exp stream on ACT (64 x [128,1024] tiles ~ 67us) is the
critical resource; it is started as early as possible and kept fed.  PE
program order: GN mms -> QKV m-tiles for pairs 0-1 -> vT(kt=0,1) -> pair 0
(injecting vT(2..7) chains into its stream) -> pair 1 (injecting pair 2's
QKV chains) -> pair 2 (injecting pair 3's) -> pair 3 -> proj.  Injected
chains reuse the score-slot PSUM banks (tag sharing) at pair head/tail
where the exp pipeline covers them.  The Exp LUT is preloaded by a dummy
activation during the QKV phase so the first real exp pays no table load.
PSUM: scores 2 slots x 2 banks, context 2 slots x 2 banks = 8 banks.

Matmul inputs are float32r (1 cycle/row for moving free >= 256; fp32 would
be 4) except probs/vt which are bf16.  f32r operands must be *produced* as
f32r, so every tile feeding a matmul is allocated f32r.
"""

import sys

sys.path.insert(0, "/opt/trn_rl_repo")

import numpy as np

B, C, HH, WW = 8, 512, 32, 32
N = HH * WW          # 1024
NH, HD = 8, 64       # heads, head dim
NG = 32              # groupnorm groups
EPS = 1e-5
NT = C // 128        # 4 channel tiles
MT = 12              # qkv m-tiles: Q 0-3 | K-padded 4-11
KT = N // 128        # 8 key tiles
NCORES = 8
LAG = 2              # context matmuls run LAG k-tiles behind scores/exp
# qkv m-tiles stored on the host in first-use order so the weight DMAs are
# two big contiguous chunks per channel tile
MPERM = (0, 4, 5, 1, 6, 7, 2, 8, 9, 3, 10, 11)
MPOS = {m: i for i, m in enumerate(MPERM)}

_CACHE: dict = {}


def _build_program():
    import concourse.bacc as bacc
    import concourse.tile as tile
    from concourse import mybir

    f32 = mybir.dt.float32
    f32r = mybir.dt.float32r
    bf16 = mybir.dt.bfloat16
    AF = mybir.ActivationFunctionType
    OP = mybir.AluOpType

    nc = bacc.Bacc("TRN2", target_bir_lowering=False, debug=False)
    # We place activation-table loads by hand (sqrt set before the GN sqrt,
    # then the combined exp+ln set once for the whole attention phase).  The
    # automatic pass does not track hand-placed loads and would re-insert a
    # single-function set load at every exp<->ln transition (8 switches,
    # ~1.3us each), so disable it for this program.
    nc.insert_act_table_loads = lambda: None

    x_d = nc.dram_tensor("x", [C, N], f32, kind="ExternalInput").ap()
    # weights in bf16: halves the weight DMA traffic and doubles the PE
    # ldweights rate (bf16 loads 1 row/cycle vs ~2.5 for f32r); the moving
    # operands stay f32r so matmuls still stream at 1 cycle/row.
    wqkv_d = nc.dram_tensor("wqkvT", [C, MT * 128], bf16, kind="ExternalInput").ap()
    wv_d = nc.dram_tensor("wvT", [C, C], bf16, kind="ExternalInput").ap()
    wproj_d = nc.dram_tensor("wprojT", [C, C], bf16, kind="ExternalInput").ap()
    bqkv_d = nc.dram_tensor("bqkv", [128, MT], f32, kind="ExternalInput").ap()
    bproj_d = nc.dram_tensor("bproj", [128, NT], f32, kind="ExternalInput").ap()
    gnw_d = nc.dram_tensor("gnw", [128, NT], f32, kind="ExternalInput").ap()
    gnb_d = nc.dram_tensor("gnb", [128, NT], f32, kind="ExternalInput").ap()
    gmask_d = nc.dram_tensor("gmask", [128, 8], f32, kind="ExternalInput").ap()
    gmaskT_d = nc.dram_tensor("gmaskT", [8, 128], f32, kind="ExternalInput").ap()
    out_d = nc.dram_tensor("out", [C, N], f32, kind="ExternalOutput").ap()

    x_dt = x_d.rearrange("(t p) n -> t p n", p=128)
    out_dt = out_d.rearrange("(t p) n -> t p n", p=128)
    wq_dt = wqkv_d.rearrange("(t p) m -> t p m", p=128)
    wv_dt = wv_d.rearrange("(t p) m -> t p m", p=128)
    wp_dt = wproj_d.rearrange("(t p) m -> t p m", p=128)

    from contextlib import ExitStack

    with tile.TileContext(nc) as tc, ExitStack() as ctx:
        sg = ctx.enter_context(tc.tile_pool(name="sg", bufs=1))
        work = ctx.enter_context(tc.tile_pool(name="work", bufs=1))
        pb_pool = ctx.enter_context(tc.tile_pool(name="pbp", bufs=8))
        small = ctx.enter_context(tc.tile_pool(name="small", bufs=4))
        outp = ctx.enter_context(tc.tile_pool(name="outp", bufs=2))
        # PSUM budget (8 banks): "sc" slots 2x2 banks (score tiles; shared by
        # the QKV/vT/proj half-accumulator chains and the GN matmuls via tag
        # reuse), "cx" slots 2x2 banks (context accumulators of a pair; also
        # chain slots outside attention).
        psc = ctx.enter_context(tc.tile_pool(name="psc", bufs=2, space="PSUM"))
        pcx = ctx.enter_context(tc.tile_pool(name="pcx", bufs=2, space="PSUM"))

        # ---- x first (critical path for GN): quarter-granular DMAs so
        # bn_stats can start on each quarter as it lands.  DMA descriptor
        # generation is ~0.5us of SERIAL work on the ISSUING engine and the
        # sync sequencer saturates if it issues everything — spread issuance
        # across sync and gpsimd (both otherwise idle here). ----
        x_sb = []
        for t in range(NT):
            xt = work.tile([128, N], f32, name=f"x{t}", tag=f"x{t}")
            eng = nc.sync if t < 2 else nc.gpsimd
            for q in range(4):
                eng.dma_start(
                    out=xt[:, q * 256 : (q + 1) * 256],
                    in_=x_dt[t][:, q * 256 : (q + 1) * 256],
                )
            x_sb.append(xt)

        # ---- small constants (biases needed at first drains), issued from
        # the scalar engine (ACT idle until the GN finalize) ----
        bqkv_sb = sg.tile([128, MT], f32, name="bqkv_sb")
        nc.scalar.dma_start(out=bqkv_sb, in_=bqkv_d)
        bproj_sb = sg.tile([128, NT], f32, name="bproj_sb")
        nc.scalar.dma_start(out=bproj_sb, in_=bproj_d)
        gnw_sb = sg.tile([128, NT], f32, name="gnw_sb")
        nc.scalar.dma_start(out=gnw_sb, in_=gnw_d)
        gnb_sb = sg.tile([128, NT], f32, name="gnb_sb")
        nc.scalar.dma_start(out=gnb_sb, in_=gnb_d)
        gmask_sb = sg.tile([128, 8], f32, name="gmask_sb")
        nc.scalar.dma_start(out=gmask_sb, in_=gmask_d)
        gmaskT_sb = sg.tile([8, 128], f32, name="gmaskT_sb")
        nc.scalar.dma_start(out=gmaskT_sb, in_=gmaskT_d)

        # ---- weights, ordered by first use.  The host stores the qkv
        # m-tiles pre-permuted into first-use order (MPERM), so each DMA is
        # one big contiguous chunk (bf16 needs >= 512B per partition row to
        # avoid the 2x small-element DMA penalty; 128-col bf16 blocks would
        # be 256B and trickle in far too slowly). ----
        wq_sb = []
        for t in range(NT):
            wt = sg.tile([128, MT * 128], bf16, name=f"wq{t}", tag=f"wq{t}")
            wq_sb.append(wt)
        for t in range(NT):  # pre-phase m-tiles (first 6 in MPERM order)
            nc.scalar.dma_start(out=wq_sb[t][:, 0:768], in_=wq_dt[t][:, 0:768])
        wv_sb = []
        for t in range(NT):
            wt = sg.tile([128, C], bf16, name=f"wv{t}", tag=f"wv{t}")
            nc.scalar.dma_start(out=wt, in_=wv_dt[t])
            wv_sb.append(wt)
        for t in range(NT):  # remaining m-tiles
            nc.scalar.dma_start(
                out=wq_sb[t][:, 768:1536], in_=wq_dt[t][:, 768:1536]
            )
        wp_sb = []
        for t in range(NT):
            wt = sg.tile([128, C], bf16, name=f"wp{t}", tag=f"wp{t}")
            nc.scalar.dma_start(out=wt, in_=wp_dt[t])
            wp_sb.append(wt)

        # vt tiles [keys, head, 64 v | 64 ones]; ones via gpsimd (idle engine)
        vt_sb = []
        for kt in range(KT):
            vt = work.tile([128, NH, 128], bf16, name=f"vt{kt}", tag=f"vt{kt}")
            nc.gpsimd.memset(vt[:, :, HD:128], 1.0)
            vt_sb.append(vt)

        eps_sb = sg.tile([8, 1], f32, name="eps_sb")
        nc.vector.memset(eps_sb, EPS)

        # ---- GroupNorm statistics ----
        allstats = sg.tile([128, 2 * NT], f32, name="allstats")
        for t in range(NT):
            bns = small.tile([128, 4, 6], f32, name=f"bns{t}", tag="bns")
            for q in range(4):
                nc.vector.bn_stats(
                    out=bns[:, q, :], in_=x_sb[t][:, q * 256 : (q + 1) * 256]
                )
            nc.vector.bn_aggr(out=allstats[:, 2 * t : 2 * t + 2], in_=bns)
            # E[x^2] = var + mean^2 into the odd column
            m2 = small.tile([128, 1], f32, name=f"m2_{t}", tag="m2")
            nc.vector.tensor_mul(
                m2, allstats[:, 2 * t : 2 * t + 1], allstats[:, 2 * t : 2 * t + 1]
            )
            nc.vector.tensor_add(
                allstats[:, 2 * t + 1 : 2 * t + 2],
                allstats[:, 2 * t + 1 : 2 * t + 2],
                m2,
            )

        # group aggregate: [8 local groups, 2*NT stats]
        grp_ps = psc.tile([8, 2 * NT], f32, name="grp_ps", tag="sc")
        nc.tensor.matmul(grp_ps, gmask_sb, allstats)
        grp_sb = sg.tile([8, 2 * NT], f32, name="grp_sb")
        nc.vector.tensor_copy(grp_sb, grp_ps)
        # var = E[x^2] - mean^2 ; rstd = 1/sqrt(var+eps)  (in cols 1::2)
        msq = sg.tile([8, NT], f32, name="msq")
        nc.vector.tensor_mul(msq, grp_sb[:, 0 : 2 * NT : 2], grp_sb[:, 0 : 2 * NT : 2])
        nc.vector.tensor_sub(
            grp_sb[:, 1 : 2 * NT : 2], grp_sb[:, 1 : 2 * NT : 2], msq
        )
        # The ONLY activation table this kernel ever loads is set 6
        # ('natural_log_exp_and_others': exp AND ln).  rstd is computed as
        # exp(-0.5*ln(var+eps)) instead of sqrt+reciprocal, so the GN, the
        # probs exps and the denominator ln/exp all run on one set: a single
        # table load at kernel start, zero switches (the baseline paid 8
        # switches, ~1.3us each).  The load is hand-emitted (the auto pass is
        # disabled) and carries no data deps, so it executes before every
        # activation — safe precisely because it is the only set.
        nc.scalar.add_instruction(
            mybir.InstLoadActFuncSet(
                name=nc.get_next_instruction_name(), ins=[], outs=[],
                act_func_set_id=6,
            )
        )
        lnv = small.tile([8, NT], f32, name="lnv", tag="lnv")
        nc.scalar.activation(
            out=lnv,
            in_=grp_sb[:, 1 : 2 * NT : 2],
            func=AF.Ln,
            bias=eps_sb,
            scale=1.0,
        )
        nc.scalar.activation(
            out=grp_sb[:, 1 : 2 * NT : 2], in_=lnv, func=AF.Exp, scale=-0.5
        )

        # broadcast group stats back to channels: [128, 2*NT]
        chan_ps = psc.tile([128, 2 * NT], f32, name="chan_ps", tag="sc")
        nc.tensor.matmul(chan_ps, gmaskT_sb, grp_sb)
        chan_sb = sg.tile([128, 2 * NT], f32, name="chan_sb")
        nc.vector.tensor_copy(chan_sb, chan_ps)

        # PE clock warm-up: the tensor engine p-state ramps with sustained
        # use (max clock only after ~3us continuous).  While the GN-apply
        # runs on DVE/GPSIMD the PE would idle and start the QKV phase at
        # half clock; stream a few throwaway matmuls on already-loaded
        # weight tiles to carry the busy streak into the QKV chains.
        for w in range(6):
            warm = psc.tile([128, 512], f32, name=f"warm{w}", tag="sc")
            nc.tensor.matmul(
                warm, wq_sb[0][:, 0:128], wq_sb[0][:, 0:512]
            )

        # A = rstd * gn_w ; Bc = gn_b - mean * A   (per channel, per tile col)
        A_sb = sg.tile([128, NT], f32, name="A_sb")
        nc.vector.tensor_mul(A_sb, chan_sb[:, 1 : 2 * NT : 2], gnw_sb)
        B_sb = sg.tile([128, NT], f32, name="B_sb")
        nc.vector.tensor_mul(B_sb, chan_sb[:, 0 : 2 * NT : 2], A_sb)
        nc.vector.tensor_sub(B_sb, gnb_sb, B_sb)

        # ---- GN apply: split across DVE and GPSIMD so the four tiles
        # finish ~2x sooner (the first QKV chains wait on xn) ----
        xn_sb = []
        for t in range(NT):
            xn = work.tile([128, N], bf16, name=f"xn{t}", tag=f"xn{t}")
            eng = nc.vector if t % 2 == 0 else nc.gpsimd
            eng.tensor_scalar(
                out=xn,
                in0=x_sb[t],
                scalar1=A_sb[:, t : t + 1],
                scalar2=B_sb[:, t : t + 1],
                op0=OP.mult,
                op1=OP.add,
            )
            xn_sb.append(xn)

        # ---- work units (each: one 4-step PSUM chain in an sc/cx slot) ----
        qkv_sb = [None] * MT

        def qkv_half(mt, hlf, pool, tag):
            if qkv_sb[mt] is None:
                qkv_sb[mt] = work.tile(
                    [128, N], bf16, name=f"qkv{mt}", tag=f"qkv{mt}"
                )
            qp = pool.tile([128, 512], f32, name=f"qp{mt}_{hlf}", tag=tag)
            mp = MPOS[mt]
            for kc in range(NT):
                nc.tensor.matmul(
                    qp,
                    wq_sb[kc][:, mp * 128 : (mp + 1) * 128],
                    xn_sb[kc][:, hlf * 512 : (hlf + 1) * 512],
                    start=(kc == 0),
                    stop=(kc == NT - 1),
                )
            nc.vector.tensor_scalar_add(
                qkv_sb[mt][:, hlf * 512 : (hlf + 1) * 512],
                qp,
                bqkv_sb[:, mt : mt + 1],
            )

        def vt_unit(kt, pool, tag):
            vp = pool.tile([128, NH, HD], f32, name=f"vp{kt}", tag=tag)
            for kc in range(NT):
                nc.tensor.matmul(
                    vp,
                    xn_sb[kc][:, kt * 128 : (kt + 1) * 128],
                    wv_sb[kc],
                    start=(kc == 0),
                    stop=(kc == NT - 1),
                )
            nc.vector.tensor_copy(vt_sb[kt][:, :, 0:HD], vp)

        # ---- attention pair with injected filler units ----
        hatt_sb = []
        for t in range(NT):
            ht = work.tile([128, N], bf16, name=f"hatt{t}", tag=f"hatt{t}")
            hatt_sb.append(ht)

        def attn_pair(j, units, prev_norm=None):
            """units: list of callables unit(pool, tag) emitting one chain.
            prev_norm: previous pair's deferred normalize — emitted after
            this pair's first scores so the Ln never stalls the in-order ACT
            queue waiting on the previous pair's last context matmul."""
            h0, h1 = 2 * j, 2 * j + 1
            cx = {}
            for h in (h0, h1):
                cx[h] = pcx.tile([128, N], f32, name=f"cx{h}", tag="cx")
            pbs = {}
            units = list(units)
            # inject points: pair head (kt=0,1) and tail (kt=8,9) by default
            # (mid-pair chains delay the score matmuls the ACT waits on).
            # Pair 0's vT units are consumed by this very pair's context
            # matmuls, so they must land just-in-time mid-pair instead:
            # vT(kt) at loop position kt-2, two steps before C(kt) needs it.
            if j == 0:
                inject_at = {0: 1, 1: 1, 4: 1, 5: 1, 6: 1, 7: 1}
            else:
                inject_at = {0: 1, 1: 1, KT: 2, KT + 1: 2}

            def emit_sc(kt):
                sc = {}
                for h in (h0, h1):
                    sc[h] = psc.tile([128, N], f32, name=f"sc{h}_{kt}", tag="sc")
                for h in (h0, h1):
                    lhsT = qkv_sb[4 + h][:, kt * 128 : (kt + 1) * 128]
                    for hlf in range(2):
                        nc.tensor.matmul(
                            sc[h][:, hlf * 512 : (hlf + 1) * 512],
                            lhsT,
                            qkv_sb[h // 2][:, hlf * 512 : (hlf + 1) * 512],
                        )
                    pb = pb_pool.tile([128, N], bf16, name=f"pb{h}_{kt}", tag="pb")
                    nc.scalar.activation(
                        out=pb, in_=sc[h], func=AF.Exp, scale=1.0 / 8.0
                    )
                    pbs[(h, kt)] = pb

            def emit_cx(kt):
                for h in (h0, h1):
                    for hlf in range(2):
                        nc.tensor.matmul(
                            cx[h][:, hlf * 512 : (hlf + 1) * 512],
                            vt_sb[kt][:, h, :],
                            pbs[(h, kt)][:, hlf * 512 : (hlf + 1) * 512],
                            start=(kt == 0),
                            stop=(kt == KT - 1),
                        )

            for kt in range(KT + LAG):
                if kt < KT:
                    emit_sc(kt)
                if kt == 1 and prev_norm is not None:
                    prev_norm()
                for _ in range(inject_at.get(kt, 0)):
                    if units:
                        units.pop(0)(psc, "sc")
                if kt >= LAG:
                    emit_cx(kt - LAG)
            while units:
                units.pop(0)(psc, "sc")

            # rows 64-127 of cx hold the softmax denominator per query
            # (vt ones block).  1/d = exp(-ln(d)) on ACT: 1 elem/lane/cycle,
            # vs ~6.3 cycles/elem for the DVE's iterative reciprocal, and the
            # combined LUT set means no table switches.  Returned as a
            # deferred closure; the caller emits it inside the NEXT pair.
            def normalize():
                for h in (h0, h1):
                    lnd = small.tile(
                        [HD, N], f32, name=f"lnd{h}", tag="lnd", bufs=2
                    )
                    nc.scalar.activation(out=lnd, in_=cx[h][HD:128, :], func=AF.Ln)
                    rsb = small.tile(
                        [HD, N], f32, name=f"rsb{h}", tag="rsb", bufs=2
                    )
                    nc.scalar.activation(out=rsb, in_=lnd, func=AF.Exp, scale=-1.0)
                    po = (h % 2) * HD
                    nc.vector.tensor_mul(
                        hatt_sb[j][po : po + HD, :], cx[h][0:HD, :], rsb
                    )

            return normalize

        # ---- PE program: pre-phase then pipelined pairs ----
        # pre: QKV m-tiles for pairs 0 and 1, then the first two vT chains
        for mt in (0, 4, 5, 1, 6, 7):
            qkv_half(mt, 0, psc, "sc")
            qkv_half(mt, 1, pcx, "cx")
        vt_unit(0, psc, "sc")
        vt_unit(1, pcx, "cx")

        def mk_vt(kt):
            return lambda pool, tag: vt_unit(kt, pool, tag)

        def mk_qkv(mt, hlf):
            return lambda pool, tag: qkv_half(mt, hlf, pool, tag)

        norm = attn_pair(0, [mk_vt(kt) for kt in range(2, KT)])
        norm = attn_pair(
            1, [mk_qkv(mt, hlf) for mt in (2, 8, 9) for hlf in range(2)], norm
        )
        norm = attn_pair(
            2, [mk_qkv(mt, hlf) for mt in (3, 10, 11) for hlf in range(2)], norm
        )
        norm = attn_pair(3, [], norm)
        norm()

        # ---- proj + bias + residual (half-N accumulators) ----
        for mt in range(NT):
            ot = outp.tile([128, N], f32, name=f"ot{mt}", tag="ot")
            for hlf in range(2):
                ppool, ptag = (psc, "sc") if hlf == 0 else (pcx, "cx")
                pp = ppool.tile([128, 512], f32, name=f"pp{mt}_{hlf}", tag=ptag)
                for kc in range(NT):
                    nc.tensor.matmul(
                        pp,
                        wp_sb[kc][:, mt * 128 : (mt + 1) * 128],
                        hatt_sb[kc][:, hlf * 512 : (hlf + 1) * 512],
                        start=(kc == 0),
                        stop=(kc == NT - 1),
                    )
                nc.vector.scalar_tensor_tensor(
                    out=ot[:, hlf * 512 : (hlf + 1) * 512],
                    in0=pp,
                    scalar=bproj_sb[:, mt : mt + 1],
                    in1=x_sb[mt][:, hlf * 512 : (hlf + 1) * 512],
                    op0=OP.add,
                    op1=OP.add,
                )
                # out DMAs: 128KB chunks fanned across queues, issued from
                # three different engines (descriptor generation is serial
                # per engine and the sync sequencer is the busiest).
                engs = (nc.sync, nc.scalar) if hlf == 0 else (nc.gpsimd, nc.sync)
                for q in range(2):
                    lo = hlf * 512 + q * 256
                    engs[q].dma_start(
                        out=out_dt[mt][:, lo : lo + 256],
                        in_=ot[:, lo : lo + 256],
                    )

    nc.compile()
    return nc


def _get_nc():
    if "nc" not in _CACHE:
        _CACHE["nc"] = _build_program()
    return _CACHE["nc"]


def _host_inputs(x, gn_w, gn_b, qkv_w, qkv_b, proj_w, proj_b):
    f32 = np.float32
    x = np.asarray(x, dtype=f32).reshape(B, C, N)
    gn_w = np.asarray(gn_w, dtype=f32)
    gn_b = np.asarray(gn_b, dtype=f32)
    qkv_w = np.asarray(qkv_w, dtype=f32)
    qkv_b = np.asarray(qkv_b, dtype=f32)
    proj_w = np.asarray(proj_w, dtype=f32)
    proj_b = np.asarray(proj_b, dtype=f32)

    # device QKV weight layout: [W_q (512 cols) | K-padded (1024 cols: per
    # head h a 128-col block, k_h placed at rows (h%2)*64, zeros elsewhere)]
    import ml_dtypes

    bf16 = ml_dtypes.bfloat16
    wq_T = qkv_w[0:512].T                      # [C, 512]
    wk_T = qkv_w[512:1024].T                   # [C, 512]
    wv_T = np.ascontiguousarray(qkv_w[1024:1536].T.astype(bf16))  # [C, 512]
    kpad = np.zeros((512, 1024), np.float32)
    for h in range(8):
        po = (h % 2) * 64
        kpad[:, h * 128 + po : h * 128 + po + 64] = wk_T[:, h * 64 : (h + 1) * 64]
    wqkv_full = np.concatenate([wq_T, kpad], axis=1)
    # permute m-tiles into first-use order (see MPERM in the kernel)
    wqkv_perm = np.concatenate(
        [wqkv_full[:, m * 128 : (m + 1) * 128] for m in MPERM], axis=1
    )
    wqkvT = np.ascontiguousarray(wqkv_perm.astype(bf16))
    # Q bias kept; K bias dropped (softmax-invariant: contributes only
    # per-query constants to the scores); V bias folded into the proj bias
    # (softmax rows sum to one, so attn(v + b_v) = attn(v) + b_v).
    bqkv_flat = np.concatenate([qkv_b[0:512], np.zeros(1024, f32)])
    bqkv = np.ascontiguousarray(bqkv_flat.reshape(MT, 128).T)
    wprojT = np.ascontiguousarray(proj_w.T.astype(bf16))
    bproj_folded = proj_b + proj_w @ qkv_b[1024:1536]
    bproj = np.ascontiguousarray(bproj_folded.astype(f32).reshape(NT, 128).T)
    gnw = np.ascontiguousarray(gn_w.reshape(NT, 128).T)
    gnb = np.ascontiguousarray(gn_b.reshape(NT, 128).T)

    p = np.arange(128)
    gmask = np.zeros((128, 8), f32)
    gmask[p, p // 16] = 1.0 / 16.0
    gmaskT = np.ascontiguousarray(
        (np.arange(128)[:, None] // 16 == np.arange(8)[None, :]).astype(f32).T
    )

    common = dict(
        wqkvT=wqkvT, wvT=wv_T, wprojT=wprojT, bqkv=bqkv, bproj=bproj,
        gnw=gnw, gnb=gnb, gmask=gmask, gmaskT=gmaskT,
    )
    return [dict(common, x=np.ascontiguousarray(x[b])) for b in range(B)]


def _run(in_maps, trace=False, **kw):
    from concourse.bass_utils import run_bass_kernel_spmd

    nc = _get_nc()
    return run_bass_kernel_spmd(nc, in_maps, list(range(NCORES)), trace=trace, **kw)


def kernel(x, gn_w, gn_b, qkv_w, qkv_b, proj_w, proj_b):
    in_maps = _host_inputs(x, gn_w, gn_b, qkv_w, qkv_b, proj_w, proj_b)
    res = _run(in_maps)
    out = np.stack([res.results[b]["out"] for b in range(B)])
    return out.reshape(B, C, HH, WW).astype(np.float32)
